# revision 13
# baseline (speedup 1.0000x reference)
"""LoRA TransformerEncoderLayer on 8 Trainium2 NeuronCores (Bass/Tile).

Sharding: sequence-parallel. The 4096 tokens (B=2 x L=2048) split into 8
shards of 512 tokens; cores 0-3 own batch 0, cores 4-7 own batch 1. Every
core holds the full (replicated) weights and computes its own 512 tokens
through the whole layer. Attention needs all 2048 keys of its batch, so
K^T and V (with an extra all-ones column that yields the softmax
denominator for free) are exchanged via one AllGather each inside the
4-core replica group. No all-reduce is needed anywhere.

On-chip layout is feature-major ("^T"): activations live as [d, t] so the
d_model contraction sits on the partition axis of every matmul. The host
pre-transposes x and pre-tiles all weights into device-friendly layouts
(host prep is not device time). LayerNorm affine (gamma/beta) is folded
into the consuming weights/biases on the host, so the device only
normalizes. Matmuls run as float32r (full PE rate at N>=256, ~1e-3 max
rel err at K=1024); everything else is fp32.
"""

import sys

sys.path.insert(0, "/opt/trn_rl_repo")

import numpy as np

import concourse.bass as bass  # noqa: F401
import concourse.mybir as mybir
import concourse.tile as tile
from concourse import bacc

D = 1024
H = 16
DH = 64
DFF = 4096
R = 8
BSZ = 2
L = 2048
NCORES = 8
T = (BSZ * L) // NCORES          # 512 tokens per core
NT = T // 128                    # 4 local token tiles
NKD = D // 128                   # 8 k-tiles over d_model
NKF = DFF // 128                 # 32 tiles over d_ff
GROUP = 4                        # cores per replica group (one batch)
NTK = GROUP * NT                 # 16 key tiles per group
LORA_SCALE = 16.0 / 8.0
EPS = 1e-5

F32 = mybir.dt.float32
F32R = mybir.dt.float32r
AF = mybir.ActivationFunctionType
OP = mybir.AluOpType

_CACHE = {}


def _unused_r(ap):
    return ap.bitcast(mybir.dt.float32r)


# --------------------------------------------------------------------------
# device program
# --------------------------------------------------------------------------

def _build(debug=False):
    nc = bacc.Bacc("TRN2", target_bir_lowering=False, debug=False,
                   num_devices=NCORES)

    def din(name, shape, dt=F32R):
        return nc.dram_tensor(name, shape, dt, kind="ExternalInput")

    tn = {
        "xt": din("xt", [D, T]),
        # pre-tiled W^T: [o_tile, ki, ko, oi]
        "wq": din("wq", [NKD, 128, NKD, 128]),
        "wk": din("wk", [NKD, 128, NKD, 128]),
        "wo": din("wo", [NKD, 128, NKD, 128]),
        "wf1": din("wf1", [NKF, 128, NKD, 128]),
        "wf2": din("wf2", [NKD, 128, NKF, 128]),
        # v weights in moving-operand chunks: [chunk, ki, ko, oi=256]
        "wv": din("wv", [4, 128, NKD, 256]),
        "aqkv": din("aqkv", [D, 3 * R]),
        "ao": din("ao", [D, R]),
        "af1": din("af1", [D, R]),
        "af2": din("af2", [DFF, R]),
        # LoRA B^T (pre-scaled by alpha/r)
        "bq": din("bq", [R, D]),
        "bk": din("bk", [R, D]),
        "bv": din("bv", [R, D]),
        "bo": din("bo", [R, D]),
        "bf1": din("bf1", [R, DFF]),
        "bf2": din("bf2", [R, D]),
        # biases, partition-major [128, n_tiles]
        "biasq": din("biasq", [128, NKD], F32),
        "biask": din("biask", [128, NKD], F32),
        "biasv": din("biasv", [128, NKD], F32),
        "biaso": din("biaso", [128, NKD], F32),
        "biasf1": din("biasf1", [128, NKF], F32),
        "biasf2": din("biasf2", [128, NKD], F32),
        "onesd": din("onesd", [128, 8, 1]),
    }
    yt = nc.dram_tensor("yt", [D, T], F32, kind="ExternalOutput")

    dbg = {}
    if debug:
        for name in ["h1", "q", "k", "ctx", "x1", "h2"]:
            dbg[name] = nc.dram_tensor("dbg_" + name, [D, T], F32,
                                       kind="ExternalOutput")

    with tile.TileContext(nc) as tc:
        _emit(nc, tc, tn, yt, dbg)

    nc.compile()
    return nc


def _emit(nc, tc, tn, yt, dbg):
    debug = bool(dbg)
    pools = []

    def pool(name, bufs, space="SBUF"):
        p = tc.tile_pool(name=name, bufs=bufs, space=space)
        pools.append(p)
        return p.__enter__()

    const = pool("const", 1)
    dram = pool("dram", 1, space="DRAM")
    big = pool("big", 4)         # 16KB slots: h1, q, ctx, h2
    gelup = pool("gelup", 1)     # 16KB gelu chunk
    x1p = pool("x1p", 1)
    accp = pool("accp", 1)
    xsp = pool("xsp", 2)         # streamed x^T k-tiles
    wpool = pool("wpool", 3)     # [128, NKD, 128] weight tiles
    wvpool = pool("wvpool", 2)   # [128, NKD, 256] v-weight chunks
    lorap = pool("lorap", 3)
    blorap = pool("blorap", 2)
    statp = pool("statp", 5)
    sqp = pool("sqp", 1)
    bcp = pool("bcp", 2)
    recp = pool("recp", 2)
    recbp = pool("recbp", 2)
    kop = pool("kop", 2)
    vsp = pool("vsp", 2)
    kvload = pool("kvload", 2)
    vaugp = pool("vaugp", 2)
    expp = pool("expp", 2)
    ystage = pool("ystage", 1)
    psc = pool("psc", 2, space="PSUM")
    pctx = pool("pctx", 2, space="PSUM")
    pgen = pool("pgen", 2, space="PSUM")

    xt_t = tn["xt"].rearrange("(ko ki) t -> ki ko t", ki=128)
    yt_t = yt.rearrange("(ko ki) t -> ki ko t", ki=128)

    # ---- constants ----
    ones8 = const.tile([128, 8, 1], F32R)
    nc.sync.dma_start(out=ones8[:], in_=tn["onesd"][:])
    ones = ones8[:, 0, :]
    eps_sb = const.tile([1, 1], F32)
    nc.vector.memset(eps_sb[:], EPS)

    def cload(name, shape):
        t = const.tile(list(shape), F32, tag=name)
        nc.sync.dma_start(out=t[:], in_=tn[name][:])
        return t

    biasq = cload("biasq", (128, NKD))
    biask = cload("biask", (128, NKD))
    biasv = cload("biasv", (128, NKD))
    biaso = cload("biaso", (128, NKD))
    biasf1 = cload("biasf1", (128, NKF))
    biasf2 = cload("biasf2", (128, NKD))

    aqkv_sb = const.tile([128, NKD, 3 * R], F32R)
    nc.sync.dma_start(out=aqkv_sb[:],
                      in_=tn["aqkv"].rearrange("(ko ki) r -> ki ko r", ki=128))
    ao_sb = const.tile([128, NKD, R], F32R)
    nc.sync.dma_start(out=ao_sb[:],
                      in_=tn["ao"].rearrange("(ko ki) r -> ki ko r", ki=128))
    af1_sb = const.tile([128, NKD, R], F32R)
    nc.sync.dma_start(out=af1_sb[:],
                      in_=tn["af1"].rearrange("(ko ki) r -> ki ko r", ki=128))
    af2_sb = const.tile([128, NKF, R], F32R)
    nc.sync.dma_start(out=af2_sb[:],
                      in_=tn["af2"].rearrange("(ko ki) r -> ki ko r", ki=128))

    # ---- pure layernorm (affine folded into consumers host-side) ----
    def layernorm(load_tile):
        """load_tile(k) -> [128, T] AP for k-tile of the input (may DMA)."""
        ps_mean = pgen.tile([128, T], F32, tag="g")
        ps_sq = pgen.tile([128, T], F32, tag="g")
        for k in range(NKD):
            xk = load_tile(k)
            sqt = sqp.tile([128, T], F32R, tag="sq")
            nc.vector.tensor_mul(sqt[:], xk, xk)
            nc.tensor.matmul(ps_mean[0:1, :], ones, (xk),
                             start=(k == 0), stop=(k == NKD - 1))
            nc.tensor.matmul(ps_sq[0:1, :], ones, (sqt[:]),
                             start=(k == 0), stop=(k == NKD - 1))
        mu = statp.tile([1, T], F32, tag="stat")
        nc.scalar.mul(mu[:], ps_mean[0:1, :], 1.0 / D)
        ex2 = statp.tile([1, T], F32, tag="stat")
        nc.scalar.mul(ex2[:], ps_sq[0:1, :], 1.0 / D)
        musq = statp.tile([1, T], F32, tag="stat")
        nc.vector.tensor_mul(musq[:], mu[:], mu[:])
        nc.vector.tensor_sub(musq[:], ex2[:], musq[:])  # now holds var
        sd = statp.tile([1, T], F32, tag="stat")
        nc.scalar.activation(sd[:], musq[:], AF.Sqrt, bias=eps_sb[:])
        rstd = statp.tile([1, T], F32, tag="stat")
        nc.vector.reciprocal(rstd[:], sd[:])
        nc.vector.tensor_mul(mu[:], mu[:], rstd[:])  # now holds mu*rstd
        a_b = bcp.tile([128, T], F32, tag="bc")
        nc.gpsimd.partition_broadcast(a_b[:], rstd[:])
        c_b = bcp.tile([128, T], F32, tag="bc")
        nc.gpsimd.partition_broadcast(c_b[:], mu[:])
        h = big.tile([128, NKD, T], F32R, tag="big")
        for k in range(NKD):
            xk = load_tile(k)
            nc.vector.tensor_mul(h[:, k, :], xk, a_b[:])
            nc.vector.tensor_sub(h[:, k, :], h[:, k, :], c_b[:])
        return h

    def xload(k):
        xs = xsp.tile([128, T], F32R, tag="xs")
        nc.sync.dma_start(out=xs[:], in_=xt_t[:, k, :])
        return xs[:]

    def dump(name, src):
        if debug:
            nc.sync.dma_start(
                out=dbg[name].rearrange("(ko ki) t -> ki ko t", ki=128)[:],
                in_=src[:].bitcast(F32))

    h1 = layernorm(xload)
    dump("h1", h1)

    # ---- LoRA-A chains for q, k, v ----
    def lora_a(a_sb, h_in, nki=NKD):
        ps = pgen.tile([R, T], F32, tag="g")
        for k in range(nki):
            nc.tensor.matmul(ps[:], (a_sb[:, k, :]), (h_in[:, k, :]),
                             start=(k == 0), stop=(k == nki - 1))
        u = lorap.tile([R, T], F32R, tag="u")
        nc.vector.tensor_copy(u[:], ps[:])
        return u

    u_q = lora_a(aqkv_sb[:, :, 0:R], h1)
    u_k = lora_a(aqkv_sb[:, :, R:2 * R], h1)
    u_v = lora_a(aqkv_sb[:, :, 2 * R:3 * R], h1)

    def proj_T(w_dram, h_in, u, b_dram, out_cb, nko=NKD):
        for j in range(nko):
            wt = wpool.tile([128, NKD, 128], F32R, tag="w")
            nc.sync.dma_start(out=wt[:], in_=w_dram[j])
            ps = pgen.tile([128, T], F32, tag="g")
            for k in range(NKD):
                nc.tensor.matmul(ps[:], (wt[:, k, :]), (h_in[:, k, :]),
                                 start=(k == 0), stop=False)
            bt = blorap.tile([R, 128], F32R, tag="b")
            nc.sync.dma_start(out=bt[:], in_=b_dram[:, j * 128:(j + 1) * 128])
            nc.tensor.matmul(ps[:], (bt[:]), (u[:]), start=False, stop=True)
            out_cb(j, ps)

    # ---- k projection -> AllGather ----
    agk_in = dram.tile([D, T], F32R)
    agk_out = dram.tile([GROUP, D, T], F32R)
    agk_in_t = agk_in.rearrange("(ko ki) t -> ki ko t", ki=128)

    def k_cb(j, ps):
        ko = kop.tile([128, T], F32R, tag="ko")
        nc.scalar.activation(ko[:], ps[:], AF.Identity, bias=biask[:, j:j + 1])
        nc.sync.dma_start(out=agk_in_t[:, j, :], in_=ko[:])
        if debug:
            nc.sync.dma_start(
                out=dbg["k"].rearrange("(ko ki) t -> ki ko t", ki=128)[:, j, :],
                in_=ko[:].bitcast(F32))

    proj_T(tn["wk"], h1, u_k, tn["bk"], k_cb)
    nc.gpsimd.collective_compute(
        "AllGather", OP.bypass, ins=[agk_in.opt()], outs=[agk_out.opt()],
        replica_groups=[[0, 1, 2, 3], [4, 5, 6, 7]])

    # ---- v projection (row layout, ones column appended) -> AllGather ----
    agv_in = dram.tile([T, H, DH + 1], F32R)
    agv_out = dram.tile([GROUP, T, H, DH + 1], F32R)

    for cc in range(4):  # 256-wide output chunks (4 heads each)
        wv = wvpool.tile([128, NKD, 256], F32R, tag="wv")
        nc.sync.dma_start(out=wv[:], in_=tn["wv"][cc])
        bvt = blorap.tile([R, 256], F32R, tag="bv")
        nc.sync.dma_start(out=bvt[:], in_=tn["bv"][:, 256 * cc:256 * (cc + 1)])
        for it in range(NT):
            ps = pgen.tile([128, 256], F32, tag="g")
            for k in range(NKD):
                nc.tensor.matmul(ps[:], (h1[:, k, 128 * it:128 * (it + 1)]),
                                 (wv[:, k, :]), start=(k == 0), stop=False)
            nc.tensor.matmul(ps[:], (u_v[:, 128 * it:128 * (it + 1)]),
                             (bvt[:]), start=False, stop=True)
            vs = vsp.tile([128, 4, DH + 1], F32R, tag="vs")
            nc.sync.dma_start(out=vs[:, :, DH:DH + 1], in_=ones8[:, 0:4, :])
            nc.scalar.activation(vs[:, :, 0:DH],
                                 ps[:].rearrange("p (h d) -> p h d", d=DH),
                                 AF.Copy)
            nc.sync.dma_start(
                out=agv_in[128 * it:128 * (it + 1), 4 * cc:4 * (cc + 1), :],
                in_=vs[:])
    nc.gpsimd.collective_compute(
        "AllGather", OP.bypass, ins=[agv_in.opt()], outs=[agv_out.opt()],
        replica_groups=[[0, 1, 2, 3], [4, 5, 6, 7]])

    # ---- q projection ----
    q_sb = big.tile([128, NKD, T], F32R, tag="big")

    def q_cb(j, ps):
        nc.scalar.activation(q_sb[:, j, :], ps[:], AF.Identity,
                             bias=biasq[:, j:j + 1])

    proj_T(tn["wq"], h1, u_q, tn["bq"], q_cb)
    dump("q", q_sb)

    # ---- attention: head pair p lives on partitions [0:64) / [64:128) ----
    ctx_sb = big.tile([128, NKD, T], F32R, tag="big")
    for p in range(NKD):
        va_e = vaugp.tile([128, NTK, DH + 1], F32R, tag="va")
        va_o = vaugp.tile([128, NTK, DH + 1], F32R, tag="va")
        nc.sync.dma_start(
            out=va_e[:],
            in_=agv_out[:, :, 2 * p, :].rearrange(
                "r (jt pp) c -> pp (r jt) c", pp=128))
        nc.sync.dma_start(
            out=va_o[:],
            in_=agv_out[:, :, 2 * p + 1, :].rearrange(
                "r (jt pp) c -> pp (r jt) c", pp=128))
        ps_ce = pctx.tile([DH + 1, T], F32, tag="c")
        ps_co = pctx.tile([DH + 1, T], F32, tag="c")
        for m in range(NTK):
            rr, jt = divmod(m, NT)
            if jt == 0:
                kpc = kvload.tile([128, T], F32R, tag="kp")
                nc.sync.dma_start(out=kpc[:],
                                  in_=agk_out[rr, 128 * p:128 * (p + 1), :])
            ps_s = psc.tile([128, 2 * T], F32, tag="s")
            nc.tensor.matmul(ps_s[:, 0:T],
                             (kpc[0:DH, 128 * jt:128 * (jt + 1)]),
                             (q_sb[0:DH, p, :]))
            nc.tensor.matmul(ps_s[:, T:2 * T],
                             (kpc[DH:128, 128 * jt:128 * (jt + 1)]),
                             (q_sb[DH:128, p, :]))
            ex = expp.tile([128, 2 * T], F32R, tag="exp")
            nc.scalar.activation(ex[:], ps_s[:], AF.Exp, scale=1.0 / 8.0)
            nc.tensor.matmul(ps_ce[:], (va_e[:, m, :]), (ex[:, 0:T]),
                             start=(m == 0), stop=(m == NTK - 1))
            nc.tensor.matmul(ps_co[:], (va_o[:, m, :]), (ex[:, T:2 * T]),
                             start=(m == 0), stop=(m == NTK - 1))
        for half, ps_c in ((0, ps_ce), (1, ps_co)):
            rec = recp.tile([1, T], F32, tag="rec")
            nc.vector.reciprocal(rec[:], ps_c[DH:DH + 1, :])
            rec_b = recbp.tile([DH, T], F32, tag="recb")
            nc.gpsimd.partition_broadcast(rec_b[:], rec[:])
            nc.vector.tensor_mul(ctx_sb[DH * half:DH * (half + 1), p, :],
                                 ps_c[0:DH, :], rec_b[:])
        nc.vector.tensor_scalar(out=ctx_sb[:, p, :], in0=ctx_sb[:, p, :],
                                scalar1=biasv[:, p:p + 1], scalar2=None,
                                op0=OP.add)
    dump("ctx", ctx_sb)

    # ---- o projection + residual ----
    u_o = lora_a(ao_sb, ctx_sb)

    x1 = x1p.tile([128, NKD, T], F32R)

    def o_cb(j, ps):
        xs = xsp.tile([128, T], F32R, tag="xs")
        nc.sync.dma_start(out=xs[:], in_=xt_t[:, j, :])
        nc.vector.scalar_tensor_tensor(out=x1[:, j, :], in0=ps[:],
                                       scalar=biaso[:, j:j + 1], in1=xs[:],
                                       op0=OP.add, op1=OP.add)

    proj_T(tn["wo"], ctx_sb, u_o, tn["bo"], o_cb)
    dump("x1", x1)

    # ---- LN2 + MLP (fc1 -> gelu -> fc2 interleaved in 4 ff chunks) ----
    h2 = layernorm(lambda k: x1[:, k, :])
    dump("h2", h2)

    u_f1 = lora_a(af1_sb, h2)

    acc = accp.tile([128, NKD, T], F32)
    ps_uf2 = pctx.tile([R, T], F32, tag="c")
    u_f2 = lorap.tile([R, T], F32R, tag="u")

    for chunk in range(4):
        gelu = gelup.tile([128, NKD, T], F32R, tag="gelu")
        for jj in range(NKD):
            j = NKD * chunk + jj
            wt = wpool.tile([128, NKD, 128], F32R, tag="w")
            nc.sync.dma_start(out=wt[:], in_=tn["wf1"][j])
            ps = pgen.tile([128, T], F32, tag="g")
            for k in range(NKD):
                nc.tensor.matmul(ps[:], (wt[:, k, :]), (h2[:, k, :]),
                                 start=(k == 0), stop=False)
            bt = blorap.tile([R, 128], F32R, tag="b")
            nc.sync.dma_start(out=bt[:], in_=tn["bf1"][:, j * 128:(j + 1) * 128])
            nc.tensor.matmul(ps[:], (bt[:]), (u_f1[:]), start=False,
                             stop=True)
            nc.scalar.activation(gelu[:, jj, :], ps[:], AF.Gelu,
                                 bias=biasf1[:, j:j + 1])
        for jj in range(NKD):  # LoRA-A for fc2 over this gelu chunk
            k = NKD * chunk + jj
            nc.tensor.matmul(ps_uf2[:], (af2_sb[:, k, :]), (gelu[:, jj, :]),
                             start=(k == 0), stop=(k == NKF - 1))
        if chunk == 3:
            nc.vector.tensor_copy(u_f2[:], ps_uf2[:])
        for j in range(NKD):  # fc2 partial products
            wt = wpool.tile([128, NKD, 128], F32R, tag="w")
            nc.sync.dma_start(
                out=wt[:],
                in_=tn["wf2"][j][:, NKD * chunk:NKD * (chunk + 1), :])
            ps = pgen.tile([128, T], F32, tag="g")
            for k in range(NKD):
                nc.tensor.matmul(ps[:], (wt[:, k, :]), (gelu[:, k, :]),
                                 start=(k == 0),
                                 stop=(chunk < 3 and k == NKD - 1))
            if chunk == 3:
                bt = blorap.tile([R, 128], F32R, tag="b")
                nc.sync.dma_start(out=bt[:],
                                  in_=tn["bf2"][:, j * 128:(j + 1) * 128])
                nc.tensor.matmul(ps[:], (bt[:]), (u_f2[:]),
                                 start=False, stop=True)
            if chunk == 0:
                nc.vector.tensor_copy(acc[:, j, :], ps[:])
            elif chunk < 3:
                nc.vector.tensor_add(acc[:, j, :], acc[:, j, :], ps[:])
            else:
                ytile = ystage.tile([128, T], F32, tag="y")
                nc.vector.tensor_add(ytile[:], acc[:, j, :], ps[:])
                nc.vector.scalar_tensor_tensor(out=ytile[:], in0=ytile[:],
                                               scalar=biasf2[:, j:j + 1],
                                               in1=x1[:, j, :],
                                               op0=OP.add, op1=OP.add)
                nc.sync.dma_start(out=yt_t[:, j, :], in_=ytile[:])

    for p in reversed(pools):
        p.__exit__(None, None, None)


# --------------------------------------------------------------------------
# host side
# --------------------------------------------------------------------------

def _tile_w(WT, nko, nki):
    """WT [d_in, d_out] -> [o_tile, ki, ko, oi] contiguous."""
    return np.ascontiguousarray(
        WT.reshape(nki, 128, nko, 128).transpose(2, 1, 0, 3))


def _prep_inputs(inputs):
    f = lambda a: np.asarray(a, np.float32)
    x = f(inputs["x"]).reshape(BSZ * L, D)
    g1, b1 = f(inputs["ln1_g"]), f(inputs["ln1_b"])
    g2, b2 = f(inputs["ln2_g"]), f(inputs["ln2_b"])

    def fold(W, b, A, Bm, g, beta):
        Wp = W * g[None, :]
        Ap = A * g[None, :]
        bp = b + W @ beta + LORA_SCALE * (Bm @ (A @ beta))
        return (Wp.astype(np.float32), bp.astype(np.float32),
                Ap.astype(np.float32))

    qW, qb, qA = fold(f(inputs["qW"]), f(inputs["qb"]), f(inputs["qA"]),
                      f(inputs["qB"]), g1, b1)
    kW, kb, kA = fold(f(inputs["kW"]), f(inputs["kb"]), f(inputs["kA"]),
                      f(inputs["kB"]), g1, b1)
    vW, vb, vA = fold(f(inputs["vW"]), f(inputs["vb"]), f(inputs["vA"]),
                      f(inputs["vB"]), g1, b1)
    f1W, f1b, f1A = fold(f(inputs["f1W"]), f(inputs["f1b"]), f(inputs["f1A"]),
                         f(inputs["f1B"]), g2, b2)
    oW, ob, oA = f(inputs["oW"]), f(inputs["ob"]), f(inputs["oA"])
    f2W, f2b, f2A = f(inputs["f2W"]), f(inputs["f2b"]), f(inputs["f2A"])

    t = lambda a: np.ascontiguousarray(a.T)
    pcol = lambda v, n: np.ascontiguousarray(v.reshape(n, 128).T)

    wvT = t(vW)  # [d_in, d_out]
    wv_tiled = np.ascontiguousarray(
        wvT.reshape(NKD, 128, 4, 256).transpose(2, 1, 0, 3))

    shared = dict(
        wq=_tile_w(t(qW), NKD, NKD), wk=_tile_w(t(kW), NKD, NKD),
        wo=_tile_w(t(oW), NKD, NKD), wf1=_tile_w(t(f1W), NKF, NKD),
        wf2=_tile_w(t(f2W), NKD, NKF), wv=wv_tiled,
        aqkv=np.ascontiguousarray(
            np.concatenate([t(qA), t(kA), t(vA)], axis=1)),
        ao=t(oA), af1=t(f1A), af2=t(f2A),
        bq=t(f(inputs["qB"])) * LORA_SCALE, bk=t(f(inputs["kB"])) * LORA_SCALE,
        bv=t(f(inputs["vB"])) * LORA_SCALE, bo=t(f(inputs["oB"])) * LORA_SCALE,
        bf1=t(f(inputs["f1B"])) * LORA_SCALE,
        bf2=t(f(inputs["f2B"])) * LORA_SCALE,
        biasq=pcol(qb, NKD), biask=pcol(kb, NKD), biasv=pcol(vb, NKD),
        biaso=pcol(ob, NKD), biasf1=pcol(f1b, NKF), biasf2=pcol(f2b, NKD),
        onesd=np.ones((128, 8, 1), np.float32),
    )
    shared = {k: np.ascontiguousarray(v, dtype=np.float32)
              for k, v in shared.items()}
    in_maps = []
    for c in range(NCORES):
        m = dict(shared)
        m["xt"] = np.ascontiguousarray(x[c * T:(c + 1) * T].T)
        in_maps.append(m)
    return in_maps


def _make_jit(nc):
    import jax
    from jax.sharding import Mesh, PartitionSpec
    from concourse import bass2jax
    try:
        from jax.experimental.shard_map import shard_map
    except ImportError:
        from jax.sharding import shard_map

    bass2jax.install_neuronx_cc_hook()
    partition_name = (nc.partition_id_tensor.name
                      if nc.partition_id_tensor else None)
    in_names, out_names, out_avals, zero_shapes = [], [], [], []
    for alloc in nc.m.functions[0].allocations:
        if not isinstance(alloc, mybir.MemoryLocationSet):
            continue
        name = alloc.memorylocations[0].name
        if alloc.kind == "ExternalInput":
            if name != partition_name:
                in_names.append(name)
        elif alloc.kind == "ExternalOutput":
            shape = tuple(alloc.tensor_shape)
            dtype = mybir.dt.np(alloc.dtype)
            out_names.append(name)
            out_avals.append(jax.core.ShapedArray(shape, dtype))
            zero_shapes.append((shape, dtype))
    n_params = len(in_names)
    all_in = list(in_names) + list(out_names)
    if partition_name is not None:
        all_in.append(partition_name)

    def _body(*args):
        operands = list(args)
        if partition_name is not None:
            operands.append(bass2jax.partition_id_tensor())
        outs = bass2jax._bass_exec_p.bind(
            *operands, out_avals=tuple(out_avals), in_names=tuple(all_in),
            out_names=tuple(out_names), lowering_input_output_aliases=(),
            sim_require_finite=True, sim_require_nnan=True, nc=nc)
        return tuple(outs)

    devices = jax.devices()[:NCORES]
    mesh = Mesh(np.asarray(devices), ("core",))
    donate = tuple(range(n_params, n_params + len(out_names)))
    fn = jax.jit(
        shard_map(_body, mesh=mesh,
                  in_specs=(PartitionSpec("core"),) * (n_params + len(out_names)),
                  out_specs=(PartitionSpec("core"),) * len(out_names),
                  check_rep=False),
        donate_argnums=donate, keep_unused=True)

    def run(in_maps):
        concat_in = [np.concatenate([np.asarray(m[nm]) for m in in_maps],
                                    axis=0)
                     for nm in in_names]
        zeros = [np.zeros((NCORES * s[0], *s[1:]), d) for (s, d) in zero_shapes]
        outs = fn(*concat_in, *zeros)
        outs = [np.asarray(o) for o in outs]
        return [{nm: outs[i].reshape(NCORES, *zero_shapes[i][0])[c]
                 for i, nm in enumerate(out_names)}
                for c in range(NCORES)]

    return run


def _get_runner(debug=False):
    key = ("runner", debug)
    if key not in _CACHE:
        nc = _build(debug=debug)
        _CACHE[key] = (nc, _make_jit(nc))
    return _CACHE[key]


def run_cores(inputs, debug=False):
    """Run the device program; returns per-core result dicts."""
    nc, run = _get_runner(debug=debug)
    return run(_prep_inputs(inputs))


def kernel(**inputs):
    results = run_cores(inputs, debug=False)
    y = np.empty((BSZ * L, D), np.float32)
    for c in range(NCORES):
        y[c * T:(c + 1) * T] = results[c]["yt"].T
    return y.reshape(BSZ, L, D)


# revision 14
# speedup vs baseline: 17.7685x; 17.7685x over previous
"""LoRA TransformerEncoderLayer on 8 Trainium2 NeuronCores (Bass/Tile).

Sharding: sequence-parallel. The 4096 tokens (B=2 x L=2048) split into 8
shards of 512 tokens; cores 0-3 own batch 0, cores 4-7 own batch 1. Every
core holds the full (replicated) weights and computes its own 512 tokens
through the whole layer. Attention needs all 2048 keys of its batch, so
K^T and V (with an extra all-ones column that yields the softmax
denominator for free) are exchanged via one AllGather each inside the
4-core replica group. No all-reduce is needed anywhere.

On-chip layout is feature-major ("^T"): activations live as [d, t] so the
d_model contraction sits on the partition axis of every matmul. The host
pre-transposes x and pre-tiles all weights into device-friendly layouts
(host prep is not device time). LayerNorm affine (gamma/beta) is folded
into the consuming weights/biases on the host, so the device only
normalizes. Matmuls run as float32r (full PE rate at N>=256, ~1e-3 max
rel err at K=1024); everything else is fp32.
"""

import sys

sys.path.insert(0, "/opt/trn_rl_repo")

import numpy as np

import concourse.bass as bass  # noqa: F401
import concourse.mybir as mybir
import concourse.tile as tile
from concourse import bacc

D = 1024
H = 16
DH = 64
DFF = 4096
R = 8
BSZ = 2
L = 2048
NCORES = 8
T = (BSZ * L) // NCORES          # 512 tokens per core
NT = T // 128                    # 4 local token tiles
NKD = D // 128                   # 8 k-tiles over d_model
NKF = DFF // 128                 # 32 tiles over d_ff
GROUP = 4                        # cores per replica group (one batch)
NTK = GROUP * NT                 # 16 key tiles per group
LORA_SCALE = 16.0 / 8.0
EPS = 1e-5

F32 = mybir.dt.float32
F32R = mybir.dt.float32r
AF = mybir.ActivationFunctionType
OP = mybir.AluOpType

_CACHE = {}


def _unused_r(ap):
    return ap.bitcast(mybir.dt.float32r)


# --------------------------------------------------------------------------
# device program
# --------------------------------------------------------------------------

def _build(debug=False):
    nc = bacc.Bacc("TRN2", target_bir_lowering=False, debug=False,
                   num_devices=NCORES)

    def din(name, shape, dt=F32R):
        return nc.dram_tensor(name, shape, dt, kind="ExternalInput")

    tn = {
        "xt": din("xt", [D, T]),
        # pre-tiled W^T: [o_tile, ki, ko, oi]
        "wq": din("wq", [NKD, 128, NKD, 128]),
        "wk": din("wk", [NKD, 128, NKD, 128]),
        "wo": din("wo", [NKD, 128, NKD, 128]),
        "wf1": din("wf1", [NKF, 128, NKD, 128]),
        "wf2": din("wf2", [NKD, 128, NKF, 128]),
        # v weights in moving-operand chunks: [chunk, ki, ko, oi=256]
        "wv": din("wv", [4, 128, NKD, 256]),
        "aqkv": din("aqkv", [D, 3 * R]),
        "ao": din("ao", [D, R]),
        "af1": din("af1", [D, R]),
        "af2": din("af2", [DFF, R]),
        # LoRA B^T (pre-scaled by alpha/r)
        "bq": din("bq", [R, D]),
        "bk": din("bk", [R, D]),
        "bv": din("bv", [R, D]),
        "bo": din("bo", [R, D]),
        "bf1": din("bf1", [R, DFF]),
        "bf2": din("bf2", [R, D]),
        # biases, partition-major [128, n_tiles]
        "biasq": din("biasq", [128, NKD], F32),
        "biask": din("biask", [128, NKD], F32),
        "biasv": din("biasv", [128, NKD], F32),
        "biaso": din("biaso", [128, NKD], F32),
        "biasf1": din("biasf1", [128, NKF], F32),
        "biasf2": din("biasf2", [128, NKD], F32),
        "onesd": din("onesd", [128, 8, 1]),
    }
    yt = nc.dram_tensor("yt", [D, T], F32, kind="ExternalOutput")

    dbg = {}
    if debug:
        for name in ["h1", "q", "k", "ctx", "x1", "h2"]:
            dbg[name] = nc.dram_tensor("dbg_" + name, [D, T], F32,
                                       kind="ExternalOutput")

    with tile.TileContext(nc) as tc:
        _emit(nc, tc, tn, yt, dbg)

    nc.compile()
    return nc


def _emit(nc, tc, tn, yt, dbg):
    debug = bool(dbg)
    pools = []

    def pool(name, bufs, space="SBUF"):
        p = tc.tile_pool(name=name, bufs=bufs, space=space)
        pools.append(p)
        return p.__enter__()

    const = pool("const", 1)
    dram = pool("dram", 1, space="DRAM")
    big = pool("big", 4)         # 16KB slots: h1, q, ctx, h2
    gelup = pool("gelup", 1)     # 16KB gelu chunk
    x1p = pool("x1p", 1)
    accp = pool("accp", 1)
    xsp = pool("xsp", 2)         # streamed x^T k-tiles
    wpool = pool("wpool", 3)     # [128, NKD, 128] weight tiles
    wvpool = pool("wvpool", 2)   # [128, NKD, 256] v-weight chunks
    lorap = pool("lorap", 3)
    blorap = pool("blorap", 2)
    statp = pool("statp", 5)
    sqp = pool("sqp", 1)
    bcp = pool("bcp", 2)
    recp = pool("recp", 2)
    recbp = pool("recbp", 2)
    kop = pool("kop", 2)
    vsp = pool("vsp", 2)
    kvload = pool("kvload", 2)
    vaugp = pool("vaugp", 2)
    expp = pool("expp", 2)
    ystage = pool("ystage", 1)
    psc = pool("psc", 2, space="PSUM")
    pctx = pool("pctx", 2, space="PSUM")
    pgen = pool("pgen", 2, space="PSUM")

    xt_t = tn["xt"].rearrange("(ko ki) t -> ki ko t", ki=128)
    yt_t = yt.rearrange("(ko ki) t -> ki ko t", ki=128)

    # ---- constants ----
    ones8 = const.tile([128, 8, 1], F32R)
    nc.sync.dma_start(out=ones8[:], in_=tn["onesd"][:])
    ones = ones8[:, 0, :]
    eps_sb = const.tile([1, 1], F32)
    nc.vector.memset(eps_sb[:], EPS)

    def cload(name, shape):
        t = const.tile(list(shape), F32, tag=name)
        nc.sync.dma_start(out=t[:], in_=tn[name][:])
        return t

    biasq = cload("biasq", (128, NKD))
    biask = cload("biask", (128, NKD))
    biasv = cload("biasv", (128, NKD))
    biaso = cload("biaso", (128, NKD))
    biasf1 = cload("biasf1", (128, NKF))
    biasf2 = cload("biasf2", (128, NKD))

    aqkv_sb = const.tile([128, NKD, 3 * R], F32R)
    nc.sync.dma_start(out=aqkv_sb[:],
                      in_=tn["aqkv"].rearrange("(ko ki) r -> ki ko r", ki=128))
    ao_sb = const.tile([128, NKD, R], F32R)
    nc.sync.dma_start(out=ao_sb[:],
                      in_=tn["ao"].rearrange("(ko ki) r -> ki ko r", ki=128))
    af1_sb = const.tile([128, NKD, R], F32R)
    nc.sync.dma_start(out=af1_sb[:],
                      in_=tn["af1"].rearrange("(ko ki) r -> ki ko r", ki=128))
    af2_sb = const.tile([128, NKF, R], F32R)
    nc.sync.dma_start(out=af2_sb[:],
                      in_=tn["af2"].rearrange("(ko ki) r -> ki ko r", ki=128))

    # ---- pure layernorm (affine folded into consumers host-side) ----
    def layernorm(load_tile):
        """load_tile(k) -> [128, T] AP for k-tile of the input (may DMA)."""
        ps_mean = pgen.tile([128, T], F32, tag="g")
        ps_sq = pgen.tile([128, T], F32, tag="g")
        for k in range(NKD):
            xk = load_tile(k)
            sqt = sqp.tile([128, T], F32R, tag="sq")
            nc.vector.tensor_mul(sqt[:], xk, xk)
            nc.tensor.matmul(ps_mean[0:1, :], ones, (xk),
                             start=(k == 0), stop=(k == NKD - 1))
            nc.tensor.matmul(ps_sq[0:1, :], ones, (sqt[:]),
                             start=(k == 0), stop=(k == NKD - 1))
        mu = statp.tile([1, T], F32, tag="stat")
        nc.scalar.mul(mu[:], ps_mean[0:1, :], 1.0 / D)
        ex2 = statp.tile([1, T], F32, tag="stat")
        nc.scalar.mul(ex2[:], ps_sq[0:1, :], 1.0 / D)
        musq = statp.tile([1, T], F32, tag="stat")
        nc.vector.tensor_mul(musq[:], mu[:], mu[:])
        nc.vector.tensor_sub(musq[:], ex2[:], musq[:])  # now holds var
        sd = statp.tile([1, T], F32, tag="stat")
        nc.scalar.activation(sd[:], musq[:], AF.Sqrt, bias=eps_sb[:])
        rstd = statp.tile([1, T], F32, tag="stat")
        nc.vector.reciprocal(rstd[:], sd[:])
        nc.vector.tensor_mul(mu[:], mu[:], rstd[:])  # now holds mu*rstd
        a_b = bcp.tile([128, T], F32, tag="bc")
        nc.gpsimd.partition_broadcast(a_b[:], rstd[:])
        c_b = bcp.tile([128, T], F32, tag="bc")
        nc.gpsimd.partition_broadcast(c_b[:], mu[:])
        h = big.tile([128, NKD, T], F32R, tag="big")
        for k in range(NKD):
            xk = load_tile(k)
            nc.vector.tensor_mul(h[:, k, :], xk, a_b[:])
            nc.vector.tensor_sub(h[:, k, :], h[:, k, :], c_b[:])
        return h

    def xload(k):
        xs = xsp.tile([128, T], F32R, tag="xs")
        nc.sync.dma_start(out=xs[:], in_=xt_t[:, k, :])
        return xs[:]

    def dump(name, src):
        if debug:
            nc.sync.dma_start(
                out=dbg[name].rearrange("(ko ki) t -> ki ko t", ki=128)[:],
                in_=src[:].bitcast(F32))

    h1 = layernorm(xload)
    dump("h1", h1)

    # ---- LoRA-A chains for q, k, v ----
    def lora_a(a_sb, h_in, nki=NKD):
        ps = pgen.tile([R, T], F32, tag="g")
        for k in range(nki):
            nc.tensor.matmul(ps[:], (a_sb[:, k, :]), (h_in[:, k, :]),
                             start=(k == 0), stop=(k == nki - 1))
        u = lorap.tile([R, T], F32R, tag="u")
        nc.vector.tensor_copy(u[:], ps[:])
        return u

    u_q = lora_a(aqkv_sb[:, :, 0:R], h1)
    u_k = lora_a(aqkv_sb[:, :, R:2 * R], h1)
    u_v = lora_a(aqkv_sb[:, :, 2 * R:3 * R], h1)

    def proj_T(w_dram, h_in, u, b_dram, out_cb, nko=NKD):
        for j in range(nko):
            wt = wpool.tile([128, NKD, 128], F32R, tag="w")
            nc.sync.dma_start(out=wt[:], in_=w_dram[j])
            ps = pgen.tile([128, T], F32, tag="g")
            for k in range(NKD):
                nc.tensor.matmul(ps[:], (wt[:, k, :]), (h_in[:, k, :]),
                                 start=(k == 0), stop=False)
            bt = blorap.tile([R, 128], F32R, tag="b")
            nc.sync.dma_start(out=bt[:], in_=b_dram[:, j * 128:(j + 1) * 128])
            nc.tensor.matmul(ps[:], (bt[:]), (u[:]), start=False, stop=True)
            out_cb(j, ps)

    # ---- k projection -> AllGather ----
    agk_in = dram.tile([D, T], F32R)
    agk_out = dram.tile([GROUP, D, T], F32R)
    agk_in_t = agk_in.rearrange("(ko ki) t -> ki ko t", ki=128)

    def k_cb(j, ps):
        ko = kop.tile([128, T], F32R, tag="ko")
        nc.scalar.activation(ko[:], ps[:], AF.Identity, bias=biask[:, j:j + 1])
        nc.sync.dma_start(out=agk_in_t[:, j, :], in_=ko[:])
        if debug:
            nc.sync.dma_start(
                out=dbg["k"].rearrange("(ko ki) t -> ki ko t", ki=128)[:, j, :],
                in_=ko[:].bitcast(F32))

    proj_T(tn["wk"], h1, u_k, tn["bk"], k_cb)
    nc.gpsimd.collective_compute(
        "AllGather", OP.bypass, ins=[agk_in.opt()], outs=[agk_out.opt()],
        replica_groups=[[0, 1, 2, 3], [4, 5, 6, 7]])

    # ---- v projection (row layout, ones column appended) -> AllGather ----
    agv_in = dram.tile([T, H, DH + 1], F32R)
    agv_out = dram.tile([GROUP, T, H, DH + 1], F32R)

    for cc in range(4):  # 256-wide output chunks (4 heads each)
        wv = wvpool.tile([128, NKD, 256], F32R, tag="wv")
        nc.sync.dma_start(out=wv[:], in_=tn["wv"][cc])
        bvt = blorap.tile([R, 256], F32R, tag="bv")
        nc.sync.dma_start(out=bvt[:], in_=tn["bv"][:, 256 * cc:256 * (cc + 1)])
        for it in range(NT):
            ps = pgen.tile([128, 256], F32, tag="g")
            for k in range(NKD):
                nc.tensor.matmul(ps[:], (h1[:, k, 128 * it:128 * (it + 1)]),
                                 (wv[:, k, :]), start=(k == 0), stop=False)
            nc.tensor.matmul(ps[:], (u_v[:, 128 * it:128 * (it + 1)]),
                             (bvt[:]), start=False, stop=True)
            vs = vsp.tile([128, 4, DH + 1], F32R, tag="vs")
            nc.sync.dma_start(out=vs[:, :, DH:DH + 1], in_=ones8[:, 0:4, :])
            nc.scalar.activation(vs[:, :, 0:DH],
                                 ps[:].rearrange("p (h d) -> p h d", d=DH),
                                 AF.Copy)
            nc.sync.dma_start(
                out=agv_in[128 * it:128 * (it + 1), 4 * cc:4 * (cc + 1), :],
                in_=vs[:])
    nc.gpsimd.collective_compute(
        "AllGather", OP.bypass, ins=[agv_in.opt()], outs=[agv_out.opt()],
        replica_groups=[[0, 1, 2, 3], [4, 5, 6, 7]])

    # ---- q projection ----
    q_sb = big.tile([128, NKD, T], F32R, tag="big")

    def q_cb(j, ps):
        nc.scalar.activation(q_sb[:, j, :], ps[:], AF.Identity,
                             bias=biasq[:, j:j + 1])

    proj_T(tn["wq"], h1, u_q, tn["bq"], q_cb)
    dump("q", q_sb)

    # ---- attention: head pair p lives on partitions [0:64) / [64:128) ----
    ctx_sb = big.tile([128, NKD, T], F32R, tag="big")
    for p in range(NKD):
        va_e = vaugp.tile([128, NTK, DH + 1], F32R, tag="va")
        va_o = vaugp.tile([128, NTK, DH + 1], F32R, tag="va")
        nc.sync.dma_start(
            out=va_e[:],
            in_=agv_out[:, :, 2 * p, :].rearrange(
                "r (jt pp) c -> pp (r jt) c", pp=128))
        nc.sync.dma_start(
            out=va_o[:],
            in_=agv_out[:, :, 2 * p + 1, :].rearrange(
                "r (jt pp) c -> pp (r jt) c", pp=128))
        ps_ce = pctx.tile([DH + 1, T], F32, tag="c")
        ps_co = pctx.tile([DH + 1, T], F32, tag="c")
        for m in range(NTK):
            rr, jt = divmod(m, NT)
            if jt == 0:
                kpc = kvload.tile([128, T], F32R, tag="kp")
                nc.sync.dma_start(out=kpc[:],
                                  in_=agk_out[rr, 128 * p:128 * (p + 1), :])
            ps_s = psc.tile([128, 2 * T], F32, tag="s")
            nc.tensor.matmul(ps_s[:, 0:T],
                             (kpc[0:DH, 128 * jt:128 * (jt + 1)]),
                             (q_sb[0:DH, p, :]))
            nc.tensor.matmul(ps_s[:, T:2 * T],
                             (kpc[DH:128, 128 * jt:128 * (jt + 1)]),
                             (q_sb[DH:128, p, :]))
            ex = expp.tile([128, 2 * T], F32R, tag="exp")
            nc.scalar.activation(ex[:], ps_s[:], AF.Exp, scale=1.0 / 8.0)
            nc.tensor.matmul(ps_ce[:], (va_e[:, m, :]), (ex[:, 0:T]),
                             start=(m == 0), stop=(m == NTK - 1))
            nc.tensor.matmul(ps_co[:], (va_o[:, m, :]), (ex[:, T:2 * T]),
                             start=(m == 0), stop=(m == NTK - 1))
        for half, ps_c in ((0, ps_ce), (1, ps_co)):
            rec = recp.tile([1, T], F32, tag="rec")
            nc.vector.reciprocal(rec[:], ps_c[DH:DH + 1, :])
            rec_b = recbp.tile([DH, T], F32, tag="recb")
            nc.gpsimd.partition_broadcast(rec_b[:], rec[:])
            nc.vector.tensor_mul(ctx_sb[DH * half:DH * (half + 1), p, :],
                                 ps_c[0:DH, :], rec_b[:])
        nc.vector.tensor_scalar(out=ctx_sb[:, p, :], in0=ctx_sb[:, p, :],
                                scalar1=biasv[:, p:p + 1], scalar2=None,
                                op0=OP.add)
    dump("ctx", ctx_sb)

    # ---- o projection + residual ----
    u_o = lora_a(ao_sb, ctx_sb)

    x1 = x1p.tile([128, NKD, T], F32R)

    def o_cb(j, ps):
        xs = xsp.tile([128, T], F32R, tag="xs")
        nc.sync.dma_start(out=xs[:], in_=xt_t[:, j, :])
        nc.vector.scalar_tensor_tensor(out=x1[:, j, :], in0=ps[:],
                                       scalar=biaso[:, j:j + 1], in1=xs[:],
                                       op0=OP.add, op1=OP.add)

    proj_T(tn["wo"], ctx_sb, u_o, tn["bo"], o_cb)
    dump("x1", x1)

    # ---- LN2 + MLP (fc1 -> gelu -> fc2 interleaved in 4 ff chunks) ----
    h2 = layernorm(lambda k: x1[:, k, :])
    dump("h2", h2)

    u_f1 = lora_a(af1_sb, h2)

    acc = accp.tile([128, NKD, T], F32)
    ps_uf2 = pctx.tile([R, T], F32, tag="c")
    u_f2 = lorap.tile([R, T], F32R, tag="u")

    for chunk in range(4):
        gelu = gelup.tile([128, NKD, T], F32R, tag="gelu")
        for jj in range(NKD):
            j = NKD * chunk + jj
            wt = wpool.tile([128, NKD, 128], F32R, tag="w")
            nc.sync.dma_start(out=wt[:], in_=tn["wf1"][j])
            ps = pgen.tile([128, T], F32, tag="g")
            for k in range(NKD):
                nc.tensor.matmul(ps[:], (wt[:, k, :]), (h2[:, k, :]),
                                 start=(k == 0), stop=False)
            bt = blorap.tile([R, 128], F32R, tag="b")
            nc.sync.dma_start(out=bt[:], in_=tn["bf1"][:, j * 128:(j + 1) * 128])
            nc.tensor.matmul(ps[:], (bt[:]), (u_f1[:]), start=False,
                             stop=True)
            nc.scalar.activation(gelu[:, jj, :], ps[:], AF.Gelu,
                                 bias=biasf1[:, j:j + 1])
        for jj in range(NKD):  # LoRA-A for fc2 over this gelu chunk
            k = NKD * chunk + jj
            nc.tensor.matmul(ps_uf2[:], (af2_sb[:, k, :]), (gelu[:, jj, :]),
                             start=(k == 0), stop=(k == NKF - 1))
        if chunk == 3:
            nc.vector.tensor_copy(u_f2[:], ps_uf2[:])
        for j in range(NKD):  # fc2 partial products
            wt = wpool.tile([128, NKD, 128], F32R, tag="w")
            nc.sync.dma_start(
                out=wt[:],
                in_=tn["wf2"][j][:, NKD * chunk:NKD * (chunk + 1), :])
            ps = pgen.tile([128, T], F32, tag="g")
            for k in range(NKD):
                nc.tensor.matmul(ps[:], (wt[:, k, :]), (gelu[:, k, :]),
                                 start=(k == 0),
                                 stop=(chunk < 3 and k == NKD - 1))
            if chunk == 3:
                bt = blorap.tile([R, 128], F32R, tag="b")
                nc.sync.dma_start(out=bt[:],
                                  in_=tn["bf2"][:, j * 128:(j + 1) * 128])
                nc.tensor.matmul(ps[:], (bt[:]), (u_f2[:]),
                                 start=False, stop=True)
            if chunk == 0:
                nc.vector.tensor_copy(acc[:, j, :], ps[:])
            elif chunk < 3:
                nc.vector.tensor_add(acc[:, j, :], acc[:, j, :], ps[:])
            else:
                ytile = ystage.tile([128, T], F32, tag="y")
                nc.vector.tensor_add(ytile[:], acc[:, j, :], ps[:])
                nc.vector.scalar_tensor_tensor(out=ytile[:], in0=ytile[:],
                                               scalar=biasf2[:, j:j + 1],
                                               in1=x1[:, j, :],
                                               op0=OP.add, op1=OP.add)
                nc.sync.dma_start(out=yt_t[:, j, :], in_=ytile[:])

    for p in reversed(pools):
        p.__exit__(None, None, None)


# --------------------------------------------------------------------------
# host side
# --------------------------------------------------------------------------

def _tile_w(WT, nko, nki):
    """WT [d_in, d_out] -> [o_tile, ki, ko, oi] contiguous."""
    return np.ascontiguousarray(
        WT.reshape(nki, 128, nko, 128).transpose(2, 1, 0, 3))


def _prep_inputs(inputs):
    f = lambda a: np.asarray(a, np.float32)
    x = f(inputs["x"]).reshape(BSZ * L, D)
    g1, b1 = f(inputs["ln1_g"]), f(inputs["ln1_b"])
    g2, b2 = f(inputs["ln2_g"]), f(inputs["ln2_b"])

    def fold(W, b, A, Bm, g, beta):
        Wp = W * g[None, :]
        Ap = A * g[None, :]
        bp = b + W @ beta + LORA_SCALE * (Bm @ (A @ beta))
        return (Wp.astype(np.float32), bp.astype(np.float32),
                Ap.astype(np.float32))

    qW, qb, qA = fold(f(inputs["qW"]), f(inputs["qb"]), f(inputs["qA"]),
                      f(inputs["qB"]), g1, b1)
    kW, kb, kA = fold(f(inputs["kW"]), f(inputs["kb"]), f(inputs["kA"]),
                      f(inputs["kB"]), g1, b1)
    vW, vb, vA = fold(f(inputs["vW"]), f(inputs["vb"]), f(inputs["vA"]),
                      f(inputs["vB"]), g1, b1)
    f1W, f1b, f1A = fold(f(inputs["f1W"]), f(inputs["f1b"]), f(inputs["f1A"]),
                         f(inputs["f1B"]), g2, b2)
    oW, ob, oA = f(inputs["oW"]), f(inputs["ob"]), f(inputs["oA"])
    f2W, f2b, f2A = f(inputs["f2W"]), f(inputs["f2b"]), f(inputs["f2A"])

    t = lambda a: np.ascontiguousarray(a.T)
    pcol = lambda v, n: np.ascontiguousarray(v.reshape(n, 128).T)

    wvT = t(vW)  # [d_in, d_out]
    wv_tiled = np.ascontiguousarray(
        wvT.reshape(NKD, 128, 4, 256).transpose(2, 1, 0, 3))

    shared = dict(
        wq=_tile_w(t(qW), NKD, NKD), wk=_tile_w(t(kW), NKD, NKD),
        wo=_tile_w(t(oW), NKD, NKD), wf1=_tile_w(t(f1W), NKF, NKD),
        wf2=_tile_w(t(f2W), NKD, NKF), wv=wv_tiled,
        aqkv=np.ascontiguousarray(
            np.concatenate([t(qA), t(kA), t(vA)], axis=1)),
        ao=t(oA), af1=t(f1A), af2=t(f2A),
        bq=t(f(inputs["qB"])) * LORA_SCALE, bk=t(f(inputs["kB"])) * LORA_SCALE,
        bv=t(f(inputs["vB"])) * LORA_SCALE, bo=t(f(inputs["oB"])) * LORA_SCALE,
        bf1=t(f(inputs["f1B"])) * LORA_SCALE,
        bf2=t(f(inputs["f2B"])) * LORA_SCALE,
        biasq=pcol(qb, NKD), biask=pcol(kb, NKD), biasv=pcol(vb, NKD),
        biaso=pcol(ob, NKD), biasf1=pcol(f1b, NKF), biasf2=pcol(f2b, NKD),
        onesd=np.ones((128, 8, 1), np.float32),
    )
    shared = {k: np.ascontiguousarray(v, dtype=np.float32)
              for k, v in shared.items()}
    in_maps = []
    for c in range(NCORES):
        m = dict(shared)
        m["xt"] = np.ascontiguousarray(x[c * T:(c + 1) * T].T)
        in_maps.append(m)
    return in_maps


def _make_jit(nc):
    import jax
    from jax.sharding import Mesh, PartitionSpec, NamedSharding
    from concourse import bass2jax
    try:
        from jax.experimental.shard_map import shard_map
    except ImportError:
        from jax.sharding import shard_map

    bass2jax.install_neuronx_cc_hook()
    partition_name = (nc.partition_id_tensor.name
                      if nc.partition_id_tensor else None)
    in_names, out_names, out_avals, zero_shapes = [], [], [], []
    for alloc in nc.m.functions[0].allocations:
        if not isinstance(alloc, mybir.MemoryLocationSet):
            continue
        name = alloc.memorylocations[0].name
        if alloc.kind == "ExternalInput":
            if name != partition_name:
                in_names.append(name)
        elif alloc.kind == "ExternalOutput":
            shape = tuple(alloc.tensor_shape)
            dtype = mybir.dt.np(alloc.dtype)
            out_names.append(name)
            out_avals.append(jax.core.ShapedArray(shape, dtype))
            zero_shapes.append((shape, dtype))
    SHARDED = {"xt"}
    n_params = len(in_names)
    all_in = list(in_names) + list(out_names)
    if partition_name is not None:
        all_in.append(partition_name)

    def _body(*args):
        operands = list(args)
        if partition_name is not None:
            operands.append(bass2jax.partition_id_tensor())
        outs = bass2jax._bass_exec_p.bind(
            *operands, out_avals=tuple(out_avals), in_names=tuple(all_in),
            out_names=tuple(out_names), lowering_input_output_aliases=(),
            sim_require_finite=True, sim_require_nnan=True, nc=nc)
        return tuple(outs)

    devices = jax.devices()[:NCORES]
    mesh = Mesh(np.asarray(devices), ("core",))
    in_specs = tuple(
        PartitionSpec("core") if nm in SHARDED else PartitionSpec()
        for nm in in_names) + (PartitionSpec("core"),) * len(out_names)
    donate = tuple(range(n_params, n_params + len(out_names)))
    fn = jax.jit(
        shard_map(_body, mesh=mesh, in_specs=in_specs,
                  out_specs=(PartitionSpec("core"),) * len(out_names),
                  check_rep=False),
        donate_argnums=donate, keep_unused=True)

    shard_sh = NamedSharding(mesh, PartitionSpec("core"))
    repl_sh = NamedSharding(mesh, PartitionSpec())

    def prepare(in_maps):
        import jax as _jax
        args = []
        for nm in in_names:
            if nm in SHARDED:
                a = np.concatenate([np.asarray(m[nm]) for m in in_maps], axis=0)
                args.append(_jax.device_put(a, shard_sh))
            else:
                args.append(_jax.device_put(np.asarray(in_maps[0][nm]), repl_sh))
        return args

    def execute(args):
        zeros = [np.zeros((NCORES * s[0], *s[1:]), d) for (s, d) in zero_shapes]
        outs = fn(*args, *zeros)
        return [np.asarray(o) for o in outs]

    def run(in_maps):
        outs = execute(prepare(in_maps))
        return [{nm: outs[i].reshape(NCORES, *zero_shapes[i][0])[c]
                 for i, nm in enumerate(out_names)}
                for c in range(NCORES)]

    run.prepare = prepare
    run.execute = execute
    run.out_names = out_names
    run.zero_shapes = zero_shapes
    return run


def _get_runner(debug=False):
    key = ("runner", debug)
    if key not in _CACHE:
        nc = _build(debug=debug)
        _CACHE[key] = (nc, _make_jit(nc))
    return _CACHE[key]


def run_cores(inputs, debug=False):
    """Run the device program; returns per-core result dicts."""
    nc, run = _get_runner(debug=debug)
    return run(_prep_inputs(inputs))


def kernel(**inputs):
    results = run_cores(inputs, debug=False)
    y = np.empty((BSZ * L, D), np.float32)
    for c in range(NCORES):
        y[c * T:(c + 1) * T] = results[c]["yt"].T
    return y.reshape(BSZ, L, D)


# revision 16
# speedup vs baseline: 146.7145x; 8.2570x over previous
"""LoRA TransformerEncoderLayer on 8 Trainium2 NeuronCores (Bass/Tile).

Sharding: sequence-parallel. The 4096 tokens (B=2 x L=2048) split into 8
shards of 512 tokens; cores 0-3 own batch 0, cores 4-7 own batch 1. Every
core holds the full (replicated) weights and computes its own 512 tokens
through the whole layer. Attention needs all 2048 keys of its batch, so
K^T and V (with an extra all-ones column that yields the softmax
denominator for free) are exchanged via one AllGather each inside the
4-core replica group. No all-reduce is needed anywhere.

On-chip layout is feature-major ("^T"): activations live as [d, t] so the
d_model contraction sits on the partition axis of every matmul. The host
pre-transposes x and pre-tiles all weights into device-friendly layouts
(host prep is not device time). LayerNorm affine (gamma/beta) is folded
into the consuming weights/biases on the host, so the device only
normalizes. Matmuls run as float32r (full PE rate at N>=256, ~1e-3 max
rel err at K=1024); everything else is fp32.
"""

import sys

sys.path.insert(0, "/opt/trn_rl_repo")

import numpy as np

import concourse.bass as bass  # noqa: F401
import concourse.mybir as mybir
import concourse.tile as tile
from concourse import bacc

D = 1024
H = 16
DH = 64
DFF = 4096
R = 8
BSZ = 2
L = 2048
NCORES = 8
T = (BSZ * L) // NCORES          # 512 tokens per core
NT = T // 128                    # 4 local token tiles
NKD = D // 128                   # 8 k-tiles over d_model
NKF = DFF // 128                 # 32 tiles over d_ff
GROUP = 4                        # cores per replica group (one batch)
NTK = GROUP * NT                 # 16 key tiles per group
LORA_SCALE = 16.0 / 8.0
EPS = 1e-5

F32 = mybir.dt.float32
F32R = mybir.dt.float32r
AF = mybir.ActivationFunctionType
OP = mybir.AluOpType

_CACHE = {}


def _unused_r(ap):
    return ap.bitcast(mybir.dt.float32r)


# --------------------------------------------------------------------------
# device program
# --------------------------------------------------------------------------

def _build(debug=False):
    nc = bacc.Bacc("TRN2", target_bir_lowering=False, debug=False,
                   num_devices=NCORES)

    def din(name, shape, dt=F32R):
        return nc.dram_tensor(name, shape, dt, kind="ExternalInput")

    tn = {
        "xt": din("xt", [D, T]),
        # pre-tiled W^T: [o_tile, ki, ko, oi]
        "wq": din("wq", [NKD, 128, NKD, 128]),
        "wk": din("wk", [NKD, 128, NKD, 128]),
        "wo": din("wo", [NKD, 128, NKD, 128]),
        "wf1": din("wf1", [NKF, 128, NKD, 128]),
        "wf2": din("wf2", [NKD, 128, NKF, 128]),
        # v weights in moving-operand chunks: [chunk, ki, ko, oi=256]
        "wv": din("wv", [4, 128, NKD, 256]),
        "aqkv": din("aqkv", [D, 3 * R]),
        "ao": din("ao", [D, R]),
        "af1": din("af1", [D, R]),
        "af2": din("af2", [DFF, R]),
        # LoRA B^T (pre-scaled by alpha/r)
        "bq": din("bq", [R, D]),
        "bk": din("bk", [R, D]),
        "bv": din("bv", [R, D]),
        "bo": din("bo", [R, D]),
        "bf1": din("bf1", [R, DFF]),
        "bf2": din("bf2", [R, D]),
        # biases, partition-major [128, n_tiles]
        "biasq": din("biasq", [128, NKD], F32),
        "biask": din("biask", [128, NKD], F32),
        "biasv": din("biasv", [128, NKD], F32),
        "biaso": din("biaso", [128, NKD], F32),
        "biasf1": din("biasf1", [128, NKF], F32),
        "biasf2": din("biasf2", [128, NKD], F32),
        "onesd": din("onesd", [128, 8, 1]),
    }
    yt = nc.dram_tensor("yt", [D, T], F32, kind="ExternalOutput")

    dbg = {}
    if debug:
        for name in ["h1", "q", "k", "ctx", "x1", "h2"]:
            dbg[name] = nc.dram_tensor("dbg_" + name, [D, T], F32,
                                       kind="ExternalOutput")

    with tile.TileContext(nc) as tc:
        _emit(nc, tc, tn, yt, dbg)

    nc.compile()
    return nc


def _emit(nc, tc, tn, yt, dbg):
    debug = bool(dbg)
    pools = []

    def pool(name, bufs, space="SBUF"):
        p = tc.tile_pool(name=name, bufs=bufs, space=space)
        pools.append(p)
        return p.__enter__()

    const = pool("const", 1)
    dram = pool("dram", 1, space="DRAM")
    big = pool("big", 4)         # 16KB slots: h1, q, ctx, h2
    gelup = pool("gelup", 1)     # 16KB gelu chunk
    x1p = pool("x1p", 1)
    accp = pool("accp", 1)
    xsp = pool("xsp", 2)         # streamed x^T k-tiles
    wpool = pool("wpool", 3)     # [128, NKD, 128] weight tiles
    wvpool = pool("wvpool", 2)   # [128, NKD, 256] v-weight chunks
    lorap = pool("lorap", 3)
    blorap = pool("blorap", 2)
    statp = pool("statp", 5)
    sqp = pool("sqp", 1)
    bcp = pool("bcp", 2)
    recp = pool("recp", 2)
    recbp = pool("recbp", 2)
    kop = pool("kop", 2)
    vsp = pool("vsp", 2)
    kvload = pool("kvload", 2)
    vaugp = pool("vaugp", 2)
    expp = pool("expp", 2)
    ystage = pool("ystage", 1)
    psc = pool("psc", 2, space="PSUM")
    pctx = pool("pctx", 2, space="PSUM")
    pgen = pool("pgen", 2, space="PSUM")

    xt_t = tn["xt"].rearrange("(ko ki) t -> ki ko t", ki=128)
    yt_t = yt.rearrange("(ko ki) t -> ki ko t", ki=128)

    # ---- constants ----
    ones8 = const.tile([128, 8, 1], F32R)
    nc.sync.dma_start(out=ones8[:], in_=tn["onesd"][:])
    ones = ones8[:, 0, :]
    eps_sb = const.tile([1, 1], F32)
    nc.vector.memset(eps_sb[:], EPS)

    def cload(name, shape):
        t = const.tile(list(shape), F32, tag=name)
        nc.sync.dma_start(out=t[:], in_=tn[name][:])
        return t

    biasq = cload("biasq", (128, NKD))
    biask = cload("biask", (128, NKD))
    biasv = cload("biasv", (128, NKD))
    biaso = cload("biaso", (128, NKD))
    biasf1 = cload("biasf1", (128, NKF))
    biasf2 = cload("biasf2", (128, NKD))

    aqkv_sb = const.tile([128, NKD, 3 * R], F32R)
    nc.sync.dma_start(out=aqkv_sb[:],
                      in_=tn["aqkv"].rearrange("(ko ki) r -> ki ko r", ki=128))
    ao_sb = const.tile([128, NKD, R], F32R)
    nc.sync.dma_start(out=ao_sb[:],
                      in_=tn["ao"].rearrange("(ko ki) r -> ki ko r", ki=128))
    af1_sb = const.tile([128, NKD, R], F32R)
    nc.sync.dma_start(out=af1_sb[:],
                      in_=tn["af1"].rearrange("(ko ki) r -> ki ko r", ki=128))
    af2_sb = const.tile([128, NKF, R], F32R)
    nc.sync.dma_start(out=af2_sb[:],
                      in_=tn["af2"].rearrange("(ko ki) r -> ki ko r", ki=128))

    # ---- pure layernorm (affine folded into consumers host-side) ----
    def layernorm(load_tile):
        """load_tile(k) -> [128, T] AP for k-tile of the input (may DMA)."""
        ps_mean = pgen.tile([128, T], F32, tag="g")
        ps_sq = pgen.tile([128, T], F32, tag="g")
        for k in range(NKD):
            xk = load_tile(k)
            sqt = sqp.tile([128, T], F32R, tag="sq")
            nc.vector.tensor_mul(sqt[:], xk, xk)
            nc.tensor.matmul(ps_mean[0:1, :], ones, (xk),
                             start=(k == 0), stop=(k == NKD - 1))
            nc.tensor.matmul(ps_sq[0:1, :], ones, (sqt[:]),
                             start=(k == 0), stop=(k == NKD - 1))
        mu = statp.tile([1, T], F32, tag="stat")
        nc.scalar.mul(mu[:], ps_mean[0:1, :], 1.0 / D)
        ex2 = statp.tile([1, T], F32, tag="stat")
        nc.scalar.mul(ex2[:], ps_sq[0:1, :], 1.0 / D)
        musq = statp.tile([1, T], F32, tag="stat")
        nc.vector.tensor_mul(musq[:], mu[:], mu[:])
        nc.vector.tensor_sub(musq[:], ex2[:], musq[:])  # now holds var
        sd = statp.tile([1, T], F32, tag="stat")
        nc.scalar.activation(sd[:], musq[:], AF.Sqrt, bias=eps_sb[:])
        rstd = statp.tile([1, T], F32, tag="stat")
        nc.vector.reciprocal(rstd[:], sd[:])
        nc.vector.tensor_mul(mu[:], mu[:], rstd[:])  # now holds mu*rstd
        a_b = bcp.tile([128, T], F32, tag="bc")
        nc.gpsimd.partition_broadcast(a_b[:], rstd[:])
        c_b = bcp.tile([128, T], F32, tag="bc")
        nc.gpsimd.partition_broadcast(c_b[:], mu[:])
        h = big.tile([128, NKD, T], F32R, tag="big")
        for k in range(NKD):
            xk = load_tile(k)
            nc.vector.tensor_mul(h[:, k, :], xk, a_b[:])
            nc.vector.tensor_sub(h[:, k, :], h[:, k, :], c_b[:])
        return h

    def xload(k):
        xs = xsp.tile([128, T], F32R, tag="xs")
        nc.sync.dma_start(out=xs[:], in_=xt_t[:, k, :])
        return xs[:]

    def dump(name, src):
        if debug:
            nc.sync.dma_start(
                out=dbg[name].rearrange("(ko ki) t -> ki ko t", ki=128)[:],
                in_=src[:].bitcast(F32))

    h1 = layernorm(xload)
    dump("h1", h1)

    # ---- LoRA-A chains for q, k, v ----
    def lora_a(a_sb, h_in, nki=NKD):
        ps = pgen.tile([R, T], F32, tag="g")
        for k in range(nki):
            nc.tensor.matmul(ps[:], (a_sb[:, k, :]), (h_in[:, k, :]),
                             start=(k == 0), stop=(k == nki - 1))
        u = lorap.tile([R, T], F32R, tag="u")
        nc.vector.tensor_copy(u[:], ps[:])
        return u

    u_q = lora_a(aqkv_sb[:, :, 0:R], h1)
    u_k = lora_a(aqkv_sb[:, :, R:2 * R], h1)
    u_v = lora_a(aqkv_sb[:, :, 2 * R:3 * R], h1)

    def proj_T(w_dram, h_in, u, b_dram, out_cb, nko=NKD):
        for j in range(nko):
            wt = wpool.tile([128, NKD, 128], F32R, tag="w")
            nc.sync.dma_start(out=wt[:], in_=w_dram[j])
            ps = pgen.tile([128, T], F32, tag="g")
            for k in range(NKD):
                nc.tensor.matmul(ps[:], (wt[:, k, :]), (h_in[:, k, :]),
                                 start=(k == 0), stop=False)
            bt = blorap.tile([R, 128], F32R, tag="b")
            nc.sync.dma_start(out=bt[:], in_=b_dram[:, j * 128:(j + 1) * 128])
            nc.tensor.matmul(ps[:], (bt[:]), (u[:]), start=False, stop=True)
            out_cb(j, ps)

    # ---- k projection -> AllGather ----
    agk_in = dram.tile([D, T], F32R)
    agk_out = dram.tile([GROUP, D, T], F32R)
    agk_in_t = agk_in.rearrange("(ko ki) t -> ki ko t", ki=128)

    def k_cb(j, ps):
        ko = kop.tile([128, T], F32R, tag="ko")
        nc.scalar.activation(ko[:], ps[:], AF.Identity, bias=biask[:, j:j + 1])
        nc.sync.dma_start(out=agk_in_t[:, j, :], in_=ko[:])
        if debug:
            nc.sync.dma_start(
                out=dbg["k"].rearrange("(ko ki) t -> ki ko t", ki=128)[:, j, :],
                in_=ko[:].bitcast(F32))

    proj_T(tn["wk"], h1, u_k, tn["bk"], k_cb)
    nc.gpsimd.collective_compute(
        "AllGather", OP.bypass, ins=[agk_in.opt()], outs=[agk_out.opt()],
        replica_groups=[[0, 1, 2, 3], [4, 5, 6, 7]])

    # ---- v projection (row layout, ones column appended) -> AllGather ----
    agv_in = dram.tile([T, H, DH + 1], F32R)
    agv_out = dram.tile([GROUP, T, H, DH + 1], F32R)

    for cc in range(4):  # 256-wide output chunks (4 heads each)
        wv = wvpool.tile([128, NKD, 256], F32R, tag="wv")
        nc.sync.dma_start(out=wv[:], in_=tn["wv"][cc])
        bvt = blorap.tile([R, 256], F32R, tag="bv")
        nc.sync.dma_start(out=bvt[:], in_=tn["bv"][:, 256 * cc:256 * (cc + 1)])
        for it in range(NT):
            ps = pgen.tile([128, 256], F32, tag="g")
            for k in range(NKD):
                nc.tensor.matmul(ps[:], (h1[:, k, 128 * it:128 * (it + 1)]),
                                 (wv[:, k, :]), start=(k == 0), stop=False)
            nc.tensor.matmul(ps[:], (u_v[:, 128 * it:128 * (it + 1)]),
                             (bvt[:]), start=False, stop=True)
            vs = vsp.tile([128, 4, DH + 1], F32R, tag="vs")
            nc.sync.dma_start(out=vs[:, :, DH:DH + 1], in_=ones8[:, 0:4, :])
            nc.scalar.activation(vs[:, :, 0:DH],
                                 ps[:].rearrange("p (h d) -> p h d", d=DH),
                                 AF.Copy)
            nc.sync.dma_start(
                out=agv_in[128 * it:128 * (it + 1), 4 * cc:4 * (cc + 1), :],
                in_=vs[:])
    nc.gpsimd.collective_compute(
        "AllGather", OP.bypass, ins=[agv_in.opt()], outs=[agv_out.opt()],
        replica_groups=[[0, 1, 2, 3], [4, 5, 6, 7]])

    # ---- q projection ----
    q_sb = big.tile([128, NKD, T], F32R, tag="big")

    def q_cb(j, ps):
        nc.scalar.activation(q_sb[:, j, :], ps[:], AF.Identity,
                             bias=biasq[:, j:j + 1])

    proj_T(tn["wq"], h1, u_q, tn["bq"], q_cb)
    dump("q", q_sb)

    # ---- attention: head pair p lives on partitions [0:64) / [64:128) ----
    ctx_sb = big.tile([128, NKD, T], F32R, tag="big")
    for p in range(NKD):
        va_e = vaugp.tile([128, NTK, DH + 1], F32R, tag="va")
        va_o = vaugp.tile([128, NTK, DH + 1], F32R, tag="va")
        nc.sync.dma_start(
            out=va_e[:],
            in_=agv_out[:, :, 2 * p, :].rearrange(
                "r (jt pp) c -> pp (r jt) c", pp=128))
        nc.sync.dma_start(
            out=va_o[:],
            in_=agv_out[:, :, 2 * p + 1, :].rearrange(
                "r (jt pp) c -> pp (r jt) c", pp=128))
        ps_ce = pctx.tile([DH + 1, T], F32, tag="c")
        ps_co = pctx.tile([DH + 1, T], F32, tag="c")
        for m in range(NTK):
            rr, jt = divmod(m, NT)
            if jt == 0:
                kpc = kvload.tile([128, T], F32R, tag="kp")
                nc.sync.dma_start(out=kpc[:],
                                  in_=agk_out[rr, 128 * p:128 * (p + 1), :])
            ps_s = psc.tile([128, 2 * T], F32, tag="s")
            nc.tensor.matmul(ps_s[:, 0:T],
                             (kpc[0:DH, 128 * jt:128 * (jt + 1)]),
                             (q_sb[0:DH, p, :]))
            nc.tensor.matmul(ps_s[:, T:2 * T],
                             (kpc[DH:128, 128 * jt:128 * (jt + 1)]),
                             (q_sb[DH:128, p, :]))
            ex = expp.tile([128, 2 * T], F32R, tag="exp")
            nc.scalar.activation(ex[:], ps_s[:], AF.Exp, scale=1.0 / 8.0)
            nc.tensor.matmul(ps_ce[:], (va_e[:, m, :]), (ex[:, 0:T]),
                             start=(m == 0), stop=(m == NTK - 1))
            nc.tensor.matmul(ps_co[:], (va_o[:, m, :]), (ex[:, T:2 * T]),
                             start=(m == 0), stop=(m == NTK - 1))
        for half, ps_c in ((0, ps_ce), (1, ps_co)):
            rec = recp.tile([1, T], F32, tag="rec")
            nc.vector.reciprocal(rec[:], ps_c[DH:DH + 1, :])
            rec_b = recbp.tile([DH, T], F32, tag="recb")
            nc.gpsimd.partition_broadcast(rec_b[:], rec[:])
            nc.vector.tensor_mul(ctx_sb[DH * half:DH * (half + 1), p, :],
                                 ps_c[0:DH, :], rec_b[:])
        nc.vector.tensor_scalar(out=ctx_sb[:, p, :], in0=ctx_sb[:, p, :],
                                scalar1=biasv[:, p:p + 1], scalar2=None,
                                op0=OP.add)
    dump("ctx", ctx_sb)

    # ---- o projection + residual ----
    u_o = lora_a(ao_sb, ctx_sb)

    x1 = x1p.tile([128, NKD, T], F32R)

    def o_cb(j, ps):
        xs = xsp.tile([128, T], F32R, tag="xs")
        nc.sync.dma_start(out=xs[:], in_=xt_t[:, j, :])
        nc.vector.scalar_tensor_tensor(out=x1[:, j, :], in0=ps[:],
                                       scalar=biaso[:, j:j + 1], in1=xs[:],
                                       op0=OP.add, op1=OP.add)

    proj_T(tn["wo"], ctx_sb, u_o, tn["bo"], o_cb)
    dump("x1", x1)

    # ---- LN2 + MLP (fc1 -> gelu -> fc2 interleaved in 4 ff chunks) ----
    h2 = layernorm(lambda k: x1[:, k, :])
    dump("h2", h2)

    u_f1 = lora_a(af1_sb, h2)

    acc = accp.tile([128, NKD, T], F32)
    ps_uf2 = pctx.tile([R, T], F32, tag="c")
    u_f2 = lorap.tile([R, T], F32R, tag="u")

    for chunk in range(4):
        gelu = gelup.tile([128, NKD, T], F32R, tag="gelu")
        for jj in range(NKD):
            j = NKD * chunk + jj
            wt = wpool.tile([128, NKD, 128], F32R, tag="w")
            nc.sync.dma_start(out=wt[:], in_=tn["wf1"][j])
            ps = pgen.tile([128, T], F32, tag="g")
            for k in range(NKD):
                nc.tensor.matmul(ps[:], (wt[:, k, :]), (h2[:, k, :]),
                                 start=(k == 0), stop=False)
            bt = blorap.tile([R, 128], F32R, tag="b")
            nc.sync.dma_start(out=bt[:], in_=tn["bf1"][:, j * 128:(j + 1) * 128])
            nc.tensor.matmul(ps[:], (bt[:]), (u_f1[:]), start=False,
                             stop=True)
            nc.scalar.activation(gelu[:, jj, :], ps[:], AF.Gelu,
                                 bias=biasf1[:, j:j + 1])
        for jj in range(NKD):  # LoRA-A for fc2 over this gelu chunk
            k = NKD * chunk + jj
            nc.tensor.matmul(ps_uf2[:], (af2_sb[:, k, :]), (gelu[:, jj, :]),
                             start=(k == 0), stop=(k == NKF - 1))
        if chunk == 3:
            nc.vector.tensor_copy(u_f2[:], ps_uf2[:])
        for j in range(NKD):  # fc2 partial products
            wt = wpool.tile([128, NKD, 128], F32R, tag="w")
            nc.sync.dma_start(
                out=wt[:],
                in_=tn["wf2"][j][:, NKD * chunk:NKD * (chunk + 1), :])
            ps = pgen.tile([128, T], F32, tag="g")
            for k in range(NKD):
                nc.tensor.matmul(ps[:], (wt[:, k, :]), (gelu[:, k, :]),
                                 start=(k == 0),
                                 stop=(chunk < 3 and k == NKD - 1))
            if chunk == 3:
                bt = blorap.tile([R, 128], F32R, tag="b")
                nc.sync.dma_start(out=bt[:],
                                  in_=tn["bf2"][:, j * 128:(j + 1) * 128])
                nc.tensor.matmul(ps[:], (bt[:]), (u_f2[:]),
                                 start=False, stop=True)
            if chunk == 0:
                nc.vector.tensor_copy(acc[:, j, :], ps[:])
            elif chunk < 3:
                nc.vector.tensor_add(acc[:, j, :], acc[:, j, :], ps[:])
            else:
                ytile = ystage.tile([128, T], F32, tag="y")
                nc.vector.tensor_add(ytile[:], acc[:, j, :], ps[:])
                nc.vector.scalar_tensor_tensor(out=ytile[:], in0=ytile[:],
                                               scalar=biasf2[:, j:j + 1],
                                               in1=x1[:, j, :],
                                               op0=OP.add, op1=OP.add)
                nc.sync.dma_start(out=yt_t[:, j, :], in_=ytile[:])

    for p in reversed(pools):
        p.__exit__(None, None, None)


# --------------------------------------------------------------------------
# host side
# --------------------------------------------------------------------------

def _tile_w(WT, nko, nki):
    """WT [d_in, d_out] -> [o_tile, ki, ko, oi] contiguous."""
    return np.ascontiguousarray(
        WT.reshape(nki, 128, nko, 128).transpose(2, 1, 0, 3))


def _prep_inputs(inputs):
    f = lambda a: np.asarray(a, np.float32)
    x = f(inputs["x"]).reshape(BSZ * L, D)
    g1, b1 = f(inputs["ln1_g"]), f(inputs["ln1_b"])
    g2, b2 = f(inputs["ln2_g"]), f(inputs["ln2_b"])

    def fold(W, b, A, Bm, g, beta):
        Wp = W * g[None, :]
        Ap = A * g[None, :]
        bp = b + W @ beta + LORA_SCALE * (Bm @ (A @ beta))
        return (Wp.astype(np.float32), bp.astype(np.float32),
                Ap.astype(np.float32))

    qW, qb, qA = fold(f(inputs["qW"]), f(inputs["qb"]), f(inputs["qA"]),
                      f(inputs["qB"]), g1, b1)
    kW, kb, kA = fold(f(inputs["kW"]), f(inputs["kb"]), f(inputs["kA"]),
                      f(inputs["kB"]), g1, b1)
    vW, vb, vA = fold(f(inputs["vW"]), f(inputs["vb"]), f(inputs["vA"]),
                      f(inputs["vB"]), g1, b1)
    f1W, f1b, f1A = fold(f(inputs["f1W"]), f(inputs["f1b"]), f(inputs["f1A"]),
                         f(inputs["f1B"]), g2, b2)
    oW, ob, oA = f(inputs["oW"]), f(inputs["ob"]), f(inputs["oA"])
    f2W, f2b, f2A = f(inputs["f2W"]), f(inputs["f2b"]), f(inputs["f2A"])

    t = lambda a: np.ascontiguousarray(a.T)
    pcol = lambda v, n: np.ascontiguousarray(v.reshape(n, 128).T)

    wvT = t(vW)  # [d_in, d_out]
    wv_tiled = np.ascontiguousarray(
        wvT.reshape(NKD, 128, 4, 256).transpose(2, 1, 0, 3))

    shared = dict(
        wq=_tile_w(t(qW), NKD, NKD), wk=_tile_w(t(kW), NKD, NKD),
        wo=_tile_w(t(oW), NKD, NKD), wf1=_tile_w(t(f1W), NKF, NKD),
        wf2=_tile_w(t(f2W), NKD, NKF), wv=wv_tiled,
        aqkv=np.ascontiguousarray(
            np.concatenate([t(qA), t(kA), t(vA)], axis=1)),
        ao=t(oA), af1=t(f1A), af2=t(f2A),
        bq=t(f(inputs["qB"])) * LORA_SCALE, bk=t(f(inputs["kB"])) * LORA_SCALE,
        bv=t(f(inputs["vB"])) * LORA_SCALE, bo=t(f(inputs["oB"])) * LORA_SCALE,
        bf1=t(f(inputs["f1B"])) * LORA_SCALE,
        bf2=t(f(inputs["f2B"])) * LORA_SCALE,
        biasq=pcol(qb, NKD), biask=pcol(kb, NKD), biasv=pcol(vb, NKD),
        biaso=pcol(ob, NKD), biasf1=pcol(f1b, NKF), biasf2=pcol(f2b, NKD),
        onesd=np.ones((128, 8, 1), np.float32),
    )
    shared = {k: np.ascontiguousarray(v, dtype=np.float32)
              for k, v in shared.items()}
    in_maps = []
    for c in range(NCORES):
        m = dict(shared)
        m["xt"] = np.ascontiguousarray(x[c * T:(c + 1) * T].T)
        in_maps.append(m)
    return in_maps


def _make_jit(nc):
    import jax
    from jax.sharding import Mesh, PartitionSpec, NamedSharding
    from concourse import bass2jax
    try:
        from jax.experimental.shard_map import shard_map
    except ImportError:
        from jax.sharding import shard_map

    bass2jax.install_neuronx_cc_hook()
    partition_name = (nc.partition_id_tensor.name
                      if nc.partition_id_tensor else None)
    in_names, out_names, out_avals, zero_shapes = [], [], [], []
    for alloc in nc.m.functions[0].allocations:
        if not isinstance(alloc, mybir.MemoryLocationSet):
            continue
        name = alloc.memorylocations[0].name
        if alloc.kind == "ExternalInput":
            if name != partition_name:
                in_names.append(name)
        elif alloc.kind == "ExternalOutput":
            shape = tuple(alloc.tensor_shape)
            dtype = mybir.dt.np(alloc.dtype)
            out_names.append(name)
            out_avals.append(jax.core.ShapedArray(shape, dtype))
            zero_shapes.append((shape, dtype))
    SHARDED = {"xt"}
    n_params = len(in_names)
    all_in = list(in_names) + list(out_names)
    if partition_name is not None:
        all_in.append(partition_name)

    def _body(*args):
        operands = list(args)
        if partition_name is not None:
            operands.append(bass2jax.partition_id_tensor())
        outs = bass2jax._bass_exec_p.bind(
            *operands, out_avals=tuple(out_avals), in_names=tuple(all_in),
            out_names=tuple(out_names), lowering_input_output_aliases=(),
            sim_require_finite=True, sim_require_nnan=True, nc=nc)
        return tuple(outs)

    devices = jax.devices()[:NCORES]
    mesh = Mesh(np.asarray(devices), ("core",))
    in_specs = tuple(
        PartitionSpec("core") if nm in SHARDED else PartitionSpec()
        for nm in in_names) + (PartitionSpec("core"),) * len(out_names)
    fn = jax.jit(
        shard_map(_body, mesh=mesh, in_specs=in_specs,
                  out_specs=(PartitionSpec("core"),) * len(out_names),
                  check_rep=False),
        keep_unused=True)

    shard_sh = NamedSharding(mesh, PartitionSpec("core"))
    repl_sh = NamedSharding(mesh, PartitionSpec())

    def prepare(in_maps):
        import jax as _jax
        args = []
        for nm in in_names:
            if nm in SHARDED:
                a = np.concatenate([np.asarray(m[nm]) for m in in_maps], axis=0)
                args.append(_jax.device_put(a, shard_sh))
            else:
                args.append(_jax.device_put(np.asarray(in_maps[0][nm]), repl_sh))
        for (s, d) in zero_shapes:
            args.append(_jax.device_put(
                np.zeros((NCORES * s[0], *s[1:]), d), shard_sh))
        return args

    def execute(args):
        return fn(*args)

    def fetch(outs):
        return [np.asarray(o) for o in outs]

    def run(in_maps):
        outs = fetch(execute(prepare(in_maps)))
        return [{nm: outs[i].reshape(NCORES, *zero_shapes[i][0])[c]
                 for i, nm in enumerate(out_names)}
                for c in range(NCORES)]

    run.prepare = prepare
    run.execute = execute
    run.fetch = fetch
    run.out_names = out_names
    run.zero_shapes = zero_shapes
    return run


def _get_runner(debug=False):
    key = ("runner", debug)
    if key not in _CACHE:
        nc = _build(debug=debug)
        _CACHE[key] = (nc, _make_jit(nc))
    return _CACHE[key]


def run_cores(inputs, debug=False):
    """Run the device program; returns per-core result dicts."""
    nc, run = _get_runner(debug=debug)
    return run(_prep_inputs(inputs))


def kernel(**inputs):
    results = run_cores(inputs, debug=False)
    y = np.empty((BSZ * L, D), np.float32)
    for c in range(NCORES):
        y[c * T:(c + 1) * T] = results[c]["yt"].T
    return y.reshape(BSZ, L, D)
